# revision 21
# baseline (speedup 1.0000x reference)
"""Causal self-attention (B=4, S=2048, D=1024, single 1024-wide head) on 8 TRN2 cores.

Sharding: core c -> batch b=c//2, parity h=c%2. The two cores of a batch
split the K^T projection by key half (core h computes keys [1024h,
1024h+1024)) and exchange halves with one pair-wise HBM AllGather, halving
the K projection work. V is projected in full on both cores: a collective
costs ~45us on the serialized CC ring (12us trigger latency + slow
transport), so a second gather could never land before the first AV phase
needs V. Each core handles the 8 query blocks {h, h+2, ..., h+14} (128 rows
each); key-extents padded to 256*(j+1) make the program identical on every
core; causality differences live in per-core additive-mask input data, not
control flow.

Phases: K^T-half projection (ec-outer over 4 concurrent PSUM groups so the
PE streams behind the initial input DMA) -> trigger gather-K -> Q^T
projection -> V projection (x^T streamed through a 2-buffer ring; the two
biggest s_phases hoisted into its tail) -> attention. Queue discipline
matters: DMA queues are FIFO, so the K bounce stores ride the scalar
engine's queue (never stuck behind bulk loads on sync), bulk loads are
issued in consumption order, and the gather readback's sync stall sits
where nothing later on that queue is needed sooner.

All matmuls run on the PE in bf16 with fp32 PSUM accumulation. Softmax skips
max-subtraction (scores are ~N(0,1); exp stays in fp32 range) so the
denominator comes free from the Exp activation's accumulate output. The
attention tail runs 2 s_phases ahead of the av_phases so the last av never
waits on an exp->transpose chain.
"""

import time

import numpy as np
import ml_dtypes

import concourse.bass as bass
import concourse.bacc as bacc
import concourse.tile as tile
from concourse import mybir
from concourse import bass_utils

BF16 = ml_dtypes.bfloat16
P = 128
B, S, D = 4, 2048, 1024
H = S // 2   # keys owned per core (half a batch)
EC = D // P  # contraction chunks (8)
NQB = 8      # query blocks per core
NKB = S // P  # key blocks per batch (16)
NCORES = 8
GROUPS = [[0, 1], [2, 3], [4, 5], [6, 7]]  # batch-pair replica groups
MASKV = -960.0  # additive pre-scale mask; -30 after the 1/sqrt(D) scale

_compiled_nc = None
_runner = None  # cached (sharded_jit, in_names, out_names, out_avals, n_params)
last_result = None  # kept for compatibility with older test harnesses


def _trace_kernel(tc, out, xT, xhT, xqT, wqT, wkT, wvT, maskadd):
    nc = tc.nc
    f32 = mybir.dt.float32
    bf16 = mybir.dt.bfloat16
    ts = bass.ts

    with (
        tc.tile_pool(name="sb", bufs=1) as sb,
        tc.tile_pool(name="dram", bufs=1, space="DRAM") as dram,
    ):
        # ---- persistent SBUF ----
        xhT_s = sb.tile([P, EC, H], bf16)   # x^T columns of own key half
        xqT_s = sb.tile([P, EC, D], bf16)   # own-query columns of x^T
        KT_s = sb.tile([P, EC, S], bf16)    # K^T full (d on partitions)
        V_s = sb.tile([P, NKB, D], bf16)    # V full, natural (s on partitions)
        QT_s = sb.tile([P, EC, D], bf16)    # Q^T for own queries
        mask_s = sb.tile([P, 2 * P], f32)   # additive mask, last 2 key tiles

        # Only K^T is exchanged: a pair-wise AllGather costs ~45us on the
        # serialized CC ring (12us trigger latency + slow transport), so a
        # second (V) gather could never land before the first AV phase. V is
        # projected in full on both cores instead.
        kin = dram.tile([EC, P, H], bf16)       # own K^T half  [d, s_local]
        kout = dram.tile([2, EC, P, H], bf16)   # gathered K^T

        # weights ring: wk -> slot0, wq -> slot1, wv -> slot0 (after the K
        # projection's last read of wk)
        def load_w(w_dram, nm):
            w_s = sb.tile([P, EC, D], bf16, tag="w", bufs=2, name=nm)
            for ec in range(EC):
                nc.sync.dma_start(w_s[:, ec], w_dram[ts(ec, P), :])
            return w_s

        # ---- input loads (sync queues are FIFO, so bulk loads are staged
        # between the stores they must not delay) ----
        # wk + xhT interleaved ec-major so the ec-outer K matmuls stream
        # right behind the DMA during the load-bound first ~12us.
        wk_s = sb.tile([P, EC, D], bf16, tag="w", bufs=2, name="wk_s")
        nc.sync.dma_start(wk_s[:, 0, :512], wkT[:P, :512])
        nc.sync.dma_start(xhT_s[:, 0, :512], xhT[:P, :512])
        nc.sync.dma_start(wk_s[:, 0, 512:], wkT[:P, 512:])
        for ec in range(1, EC):
            nc.sync.dma_start(wk_s[:, ec], wkT[ts(ec, P), :])
            nc.sync.dma_start(xhT_s[:, ec, :512], xhT[ts(ec, P), :512])
        for ec in range(EC):
            nc.sync.dma_start(xhT_s[:, ec, 512:], xhT[ts(ec, P), 512:])
        nc.sync.dma_start(mask_s, maskadd)

        # One PSUM pool for the whole kernel ("s" ring 4 banks + "big" ring
        # 4 banks): closing a scoped pool mid-kernel acts as a coarse
        # barrier on every pending accumulator copy (~10us PE stall).
        with tc.tile_pool(name="ps", bufs=2, space="PSUM") as ps:
            # ---- K^T half projection: KTh[d,s] = sum_e wkT[e,d]*xhT[e,s] ----
            # ec-outer over 4 concurrent PSUM groups: each arriving (wk, xhT)
            # ec-chunk feeds 4 matmuls, so the PE streams behind the initial
            # DMA instead of idling ~7us.
            def k_chunk(sc, dh):
                kaccs = [ps.tile([P, 512], f32, tag="s", bufs=4,
                                 name=f"kb{sc}_{dh}_{i}") for i in range(4)]
                for ec in range(EC):
                    for i in range(4):
                        nc.tensor.matmul(
                            kaccs[i], wk_s[:, ec, ts(4 * dh + i, P)],
                            xhT_s[:, ec, ts(sc, 512)],
                            start=(ec == 0), stop=(ec == EC - 1))
                for i in range(4):
                    kst = sb.tile([P, 512], bf16, tag="kst", bufs=3)
                    # alternate copy engines (DVE is idle here) so the 4
                    # serial copies at each chunk boundary take 2 slots, not 4
                    if i % 2 == 0:
                        nc.scalar.copy(kst, kaccs[i])
                        nc.scalar.dma_start(kin[4 * dh + i, :, ts(sc, 512)], kst)
                    else:
                        nc.vector.tensor_copy(kst, kaccs[i])
                        nc.scalar.dma_start(kin[4 * dh + i, :, ts(sc, 512)], kst)

            # QT-phase loads issue before the K chunks run; the kin stores
            # go out on the scalar engine's queue so they never sit behind
            # these bulk loads
            wq_s = load_w(wqT, "wq_s")
            for ec in range(EC):
                nc.sync.dma_start(xqT_s[:, ec], xqT[ts(ec, P), :])
            k_chunk(0, 0)
            k_chunk(0, 1)
            k_chunk(1, 0)
            k_chunk(1, 1)

            nc.gpsimd.collective_compute(
                "AllGather", mybir.AluOpType.bypass, replica_groups=GROUPS,
                ins=[kin[:]], outs=[kout[:]])

            # V-phase loads: x^T streamed in 4 column-quads through a ring
            # (the V projection is done on both cores of a pair -- cheaper
            # than a second collective). Quad loads 2+ stall on the ring
            # until the V loop frees a slot; that also paces the rbK stall.
            wv_s = load_w(wvT, "wv_s")
            xv_t = [None] * 4

            def load_xv(q):
                xv = sb.tile([P, EC, 512], bf16, tag="xv", bufs=2,
                             name=f"xv{q}")
                for ec in range(EC):
                    nc.sync.dma_start(xv[:, ec],
                                      xT[ts(ec, P), 512 * q:512 * (q + 1)])
                xv_t[q] = xv

            # quads 0/1 preloaded here; 2/3 are traced inside the V loop so
            # the ring reuse (2 bufs) serializes correctly behind the reads
            load_xv(0)
            load_xv(1)

            # readback: stalls sync until ccK completes (~85us); the hoisted
            # s_phases below need K^T only from ~120us
            for g in range(2):
                for dc in range(EC):
                    nc.sync.dma_start(KT_s[:, dc, g * H:(g + 1) * H],
                                      kout[g, dc])

            # ---- Q^T projection: QT[d, q] = sum_e WqT[e, d] * xqT[e, q] ----
            for dc in range(EC):
                for nh in range(2):
                    acch = ps.tile([P, 512], f32, tag="s", bufs=4)
                    for ec in range(EC):
                        nc.tensor.matmul(
                            acch, wq_s[:, ec, ts(dc, P)],
                            xqT_s[:, ec, ts(nh, 512)],
                            start=(ec == 0), stop=(ec == EC - 1))
                    nc.scalar.copy(QT_s[:, dc, ts(nh, 512)], acch)

            # ---- attention, one 128-row query block at a time ----
            # Software-pipelined: S/exp of the NEXT block is traced between the
            # S/exp and transpose/AV of the current one, so the PE has matmul
            # work while ACT/DVE chew through exp and P^T copies.
            inv_sqrt_d = 1.0 / float(np.sqrt(D))

            def s_phase(j):
                nkt = 2 * j + 2          # key tiles (uniform across cores)
                ncols = nkt * P
                nch = (ncols + 511) // 512
                p_sb = sb.tile([P, S], bf16, tag="p_sb", bufs=3)
                pT_sb = sb.tile([P, NKB, P], bf16, tag="pT_sb", bufs=3)
                dsl = sb.tile([P, 4], f32, tag="dsl", bufs=3)
                for ch in range(nch):
                    c0 = ch * 512
                    cw = min(512, ncols - c0)
                    sfull = ps.tile([P, 512], f32, tag="s", bufs=4)
                    sps = sfull[:, :cw]
                    for dc in range(EC):
                        nc.tensor.matmul(
                            sps, QT_s[:, dc, ts(j, P)], KT_s[:, dc, c0:c0 + cw],
                            start=(dc == 0), stop=(dc == EC - 1))
                    if c0 + cw == ncols:  # last chunk holds the 2 maskable tiles
                        nc.vector.tensor_add(
                            sps[:, cw - 2 * P:cw], sps[:, cw - 2 * P:cw], mask_s)
                    nc.scalar.activation(
                        p_sb[:, c0:c0 + cw], sps,
                        mybir.ActivationFunctionType.Exp,
                        scale=inv_sqrt_d,
                        accum_out=dsl[:, ch:ch + 1])
                    # xbar-transpose the finished chunk off the hot engines:
                    # pT_sb[p, kt, q] = p_sb[q, 128*kt + p]
                    nc.sync.dma_start(pT_sb[:, ch * 4:ch * 4 + cw // P, :],
                                      p_sb[:, c0:c0 + cw], transpose=True)
                return p_sb, pT_sb, dsl, nkt, nch

            def av_phase(j, p_sb, pT_sb, dsl, nkt, nch, nsplit=2):
                denom = sb.tile([P, 1], f32, tag="den", bufs=2)
                nc.vector.reduce_sum(denom, dsl[:, :nch], axis=mybir.AxisListType.X)
                recip = sb.tile([P, 1], f32, tag="rcp", bufs=2)
                nc.vector.reciprocal(recip, denom)

                acc = ps.tile([P, D], f32, tag="big")
                for kt in range(nkt):
                    for nh in range(2):
                        nc.tensor.matmul(
                            acc[:, ts(nh, 512)], pT_sb[:, kt, :],
                            V_s[:, kt, ts(nh, 512)],
                            start=(kt == 0), stop=(kt == nkt - 1))
                o_sb = sb.tile([P, D], f32, tag="o_sb", bufs=2)
                # normalize on DVE (idle now), split so the out DMA overlaps
                w = D // nsplit
                for i in range(nsplit):
                    nc.vector.tensor_scalar_mul(
                        o_sb[:, i * w:(i + 1) * w], acc[:, i * w:(i + 1) * w],
                        recip)
                    nc.sync.dma_start(out[j, :, i * w:(i + 1) * w],
                                      o_sb[:, i * w:(i + 1) * w])

            # ---- V projection (full batch, duplicated on the pair) ----
            # V[s, d] = sum_e xT[e, s] * WvT[e, d]; the two biggest s_phases
            # are hoisted into the tail of this loop (after the K^T readback
            # has landed) so their exp/transpose latencies hide under V
            # matmuls and the first av_phase can start right at V's end.
            v_hoist = {}
            for kb in range(NKB):
                if kb == 4:
                    load_xv(2)
                elif kb == 8:
                    load_xv(3)
                xv = xv_t[kb // 4]
                acc = ps.tile([P, D], f32, tag="big")
                for ec in range(EC):
                    lhsT = xv[:, ec, ts(kb % 4, P)]
                    for nh in range(2):
                        nc.tensor.matmul(
                            acc[:, ts(nh, 512)], lhsT, wv_s[:, ec, ts(nh, 512)],
                            start=(ec == 0), stop=(ec == EC - 1))
                nc.vector.tensor_copy(V_s[:, kb], acc)
                if kb == 12:
                    v_hoist[7] = s_phase(7)
                elif kb == 14:
                    v_hoist[6] = s_phase(6)

            # big first; small blocks interleaved late, with the tail run
            # 2 s_phases ahead (3-deep p/pT rings) so the last av-phases
            # never wait on an exp->transpose chain
            states = dict(v_hoist)

            def run_s(j):
                states[j] = s_phase(j)

            def run_av(j, nsplit=4):
                av_phase(j, *states.pop(j), nsplit=nsplit)

            run_av(7); run_s(5); run_av(6); run_s(0); run_av(5); run_s(1)
            run_av(0); run_s(2); run_s(3); run_av(1); run_s(4); run_av(2)
            run_av(3); run_av(4, nsplit=4)


def build_nc(debug=False):
    nc = bacc.Bacc("TRN2", target_bir_lowering=False, debug=debug,
                   enable_asserts=False, num_devices=NCORES)
    bf16 = mybir.dt.bfloat16
    f32 = mybir.dt.float32
    xT = nc.dram_tensor("xT", (D, S), bf16, kind="ExternalInput").ap()
    xhT = nc.dram_tensor("xhT", (D, H), bf16, kind="ExternalInput").ap()
    xqT = nc.dram_tensor("xqT", (D, D), bf16, kind="ExternalInput").ap()
    wqT = nc.dram_tensor("wqT", (D, D), bf16, kind="ExternalInput").ap()
    wkT = nc.dram_tensor("wkT", (D, D), bf16, kind="ExternalInput").ap()
    wvT = nc.dram_tensor("wvT", (D, D), bf16, kind="ExternalInput").ap()
    maskadd = nc.dram_tensor("maskadd", (P, 2 * P), f32,
                             kind="ExternalInput").ap()
    out = nc.dram_tensor("out", (NQB, P, D), f32, kind="ExternalOutput").ap()
    with tile.TileContext(nc) as tc:
        _trace_kernel(tc, out, xT, xhT, xqT, wqT, wkT, wvT, maskadd)
    nc.compile()
    return nc


def _get_compiled():
    global _compiled_nc
    if _compiled_nc is None:
        _compiled_nc = build_nc(debug=False)
    return _compiled_nc


def _get_runner():
    """Jit-once shard_map runner over the 8 NeuronCores.

    Mirrors bass2jax.run_bass_via_pjrt's multi-core branch, but caches the
    jitted executable so repeat kernel() calls skip retracing/recompiling.
    """
    global _runner
    if _runner is not None:
        return _runner
    import jax
    from jax.experimental.shard_map import shard_map
    from jax.sharding import Mesh, PartitionSpec
    from concourse import bass2jax

    nc = _get_compiled()
    bass2jax.install_neuronx_cc_hook()

    partition_name = (nc.partition_id_tensor.name
                      if nc.partition_id_tensor else None)
    in_names, out_names, out_avals, zero_outs = [], [], [], []
    for alloc in nc.m.functions[0].allocations:
        if not isinstance(alloc, mybir.MemoryLocationSet):
            continue
        name = alloc.memorylocations[0].name
        if alloc.kind == "ExternalInput":
            if name != partition_name:
                in_names.append(name)
        elif alloc.kind == "ExternalOutput":
            shape = tuple(alloc.tensor_shape)
            dtype = mybir.dt.np(alloc.dtype)
            out_names.append(name)
            out_avals.append(jax.core.ShapedArray(shape, dtype))
            zero_outs.append(np.zeros(shape, dtype))
    n_params = len(in_names)
    all_in_names = list(in_names) + list(out_names)
    if partition_name is not None:
        all_in_names.append(partition_name)
    donate = tuple(range(n_params, n_params + len(out_names)))

    def _body(*args):
        operands = list(args)
        if partition_name is not None:
            operands.append(bass2jax.partition_id_tensor())
        outs = bass2jax._bass_exec_p.bind(
            *operands,
            out_avals=tuple(out_avals),
            in_names=tuple(all_in_names),
            out_names=tuple(out_names),
            lowering_input_output_aliases=(),
            sim_require_finite=True,
            sim_require_nnan=True,
            nc=nc,
        )
        return tuple(outs)

    devices = jax.devices()[:NCORES]
    mesh = Mesh(np.asarray(devices), ("core",))
    nin = n_params + len(out_names)
    sharded = jax.jit(
        shard_map(_body, mesh=mesh,
                  in_specs=(PartitionSpec("core"),) * nin,
                  out_specs=(PartitionSpec("core"),) * len(out_names),
                  check_rep=False),
        donate_argnums=donate, keep_unused=True)
    _runner = (sharded, in_names, out_names, out_avals, n_params, zero_outs, mesh)
    return _runner


def run_device(in_maps):
    """Execute the compiled NEFF on all 8 cores; returns per-core output dicts."""
    sharded, in_names, out_names, out_avals, n_params, zero_outs, _ = _get_runner()
    concat_in = [
        np.concatenate([np.asarray(in_maps[c][nm]) for c in range(NCORES)], axis=0)
        for nm in in_names
    ]
    concat_zeros = [
        np.zeros((NCORES * z.shape[0], *z.shape[1:]), z.dtype) for z in zero_outs
    ]
    out_arrs = sharded(*concat_in, *concat_zeros)
    return [
        {nm: np.asarray(out_arrs[i]).reshape(NCORES, *out_avals[i].shape)[c]
         for i, nm in enumerate(out_names)}
        for c in range(NCORES)
    ]


def make_in_maps(x):
    """Per-core host-side slicing + layout prep (no matmul math here)."""
    x = np.asarray(x, dtype=np.float32)
    r = np.arange(P)
    tri_add = np.where(r[None, :] <= r[:, None], 0.0, MASKV).astype(np.float32)
    mask_h = []
    for h in range(2):
        if h == 0:
            blk = np.concatenate(
                [tri_add, np.full((P, P), MASKV, np.float32)], axis=1)
        else:
            blk = np.concatenate([np.zeros((P, P), np.float32), tri_add], axis=1)
        mask_h.append(np.ascontiguousarray(blk).astype(np.float32))

    in_maps = []
    xT_b = {}
    for c in range(NCORES):
        b, h = c // 2, c % 2
        if b not in xT_b:
            xT_b[b] = np.ascontiguousarray(x[b].T).astype(BF16)
        blocks = [2 * j + h for j in range(NQB)]
        xq = np.concatenate([x[b][g * P:(g + 1) * P] for g in blocks], axis=0)
        xqT = np.ascontiguousarray(xq.T).astype(BF16)
        in_maps.append({
            "xT": xT_b[b],
            "xhT": np.ascontiguousarray(xT_b[b][:, h * H:(h + 1) * H]),
            "xqT": xqT,
            "maskadd": mask_h[h],
        })
    return in_maps


def make_weight_map(inputs):
    """Pre-transposed bf16 weights keyed by NEFF input name."""
    return {
        "wqT": np.ascontiguousarray(np.asarray(inputs["Wq"], np.float32).T).astype(BF16),
        "wkT": np.ascontiguousarray(np.asarray(inputs["Wk"], np.float32).T).astype(BF16),
        "wvT": np.ascontiguousarray(np.asarray(inputs["Wv"], np.float32).T).astype(BF16),
    }


def kernel(x, Wq, bq, Wk, bk, Wv, bv, mask):
    global last_result
    x = np.asarray(x, np.float32)
    Wq = np.asarray(Wq, np.float32)
    Wk = np.asarray(Wk, np.float32)
    Wv = np.asarray(Wv, np.float32)
    bq = np.asarray(bq, np.float32)
    bk = np.asarray(bk, np.float32)
    bv = np.asarray(bv, np.float32)
    mask = np.asarray(mask)

    causal = bool(np.array_equal(mask != 0, np.tril(np.ones(mask.shape, bool))))
    if np.any(bq) or np.any(bk) or not causal:
        return _np_reference(x, Wq, bq, Wk, bk, Wv, bv, mask)

    in_maps = make_in_maps(x)
    wT = {
        "wqT": np.ascontiguousarray(Wq.T).astype(BF16),
        "wkT": np.ascontiguousarray(Wk.T).astype(BF16),
        "wvT": np.ascontiguousarray(Wv.T).astype(BF16),
    }
    for m in in_maps:
        m.update(wT)

    results = None
    for attempt in range(3):  # remote NeuronCores occasionally wedge transiently
        try:
            results = run_device(in_maps)
            break
        except Exception:
            if attempt == 2:
                raise
            time.sleep(30)

    out = np.empty((B * S, D), np.float32)
    for c in range(NCORES):
        b, h = c // 2, c % 2
        o = np.asarray(results[c]["out"], np.float32)
        for j in range(NQB):
            g = 2 * j + h
            out[b * S + g * P: b * S + (g + 1) * P] = o[j]
    if np.any(bv):
        out = out + bv[None, :]  # attn rows sum to 1, so bv adds exactly
    return out


def _np_reference(x, Wq, bq, Wk, bk, Wv, bv, mask):
    outs = []
    for b in range(x.shape[0]):
        xb = x[b]
        Q = xb @ Wq.T + bq
        K = xb @ Wk.T + bk
        V = xb @ Wv.T + bv
        Sc = (Q @ K.T) / np.float32(np.sqrt(x.shape[2]))
        Sc = np.where(mask == 0, np.float32(-1e9), Sc)
        Sc = Sc - Sc.max(axis=1, keepdims=True)
        E = np.exp(Sc)
        A = E / E.sum(axis=1, keepdims=True)
        outs.append(A @ V)
    return np.concatenate(outs, axis=0).astype(np.float32)


# revision 23
# speedup vs baseline: 1.0479x; 1.0479x over previous
"""Causal self-attention (B=4, S=2048, D=1024, single 1024-wide head) on 8 TRN2 cores.

Sharding: core c -> batch b=c//2, parity h=c%2. The two cores of a batch
split the K^T projection by key half (core h computes keys [1024h,
1024h+1024)) and exchange halves with one pair-wise HBM AllGather, halving
the K projection work. V is projected in full on both cores: a collective
costs ~45us on the serialized CC ring (12us trigger latency + slow
transport), so a second gather could never land before the first AV phase
needs V. Each core handles the 8 query blocks {h, h+2, ..., h+14} (128 rows
each); key-extents padded to 256*(j+1) make the program identical on every
core; causality differences live in per-core additive-mask input data, not
control flow.

Phases: K^T-half projection (ec-outer over 4 concurrent PSUM groups so the
PE streams behind the initial input DMA) -> trigger gather-K -> Q^T
projection -> V projection (x^T streamed through a 2-buffer ring; the two
biggest s_phases hoisted into its tail) -> attention. Queue discipline
matters: DMA queues are FIFO, so the K bounce stores ride the scalar
engine's queue (never stuck behind bulk loads on sync), bulk loads are
issued in consumption order, and the gather readback's sync stall sits
where nothing later on that queue is needed sooner.

All matmuls run on the PE in bf16 with fp32 PSUM accumulation. Softmax skips
max-subtraction (scores are ~N(0,1); exp stays in fp32 range) so the
denominator comes free from the Exp activation's accumulate output. The
attention tail runs 2 s_phases ahead of the av_phases so the last av never
waits on an exp->transpose chain.
"""

import time

import numpy as np
import ml_dtypes

import concourse.bass as bass
import concourse.bacc as bacc
import concourse.tile as tile
from concourse import mybir
from concourse import bass_utils

BF16 = ml_dtypes.bfloat16
P = 128
B, S, D = 4, 2048, 1024
H = S // 2   # keys owned per core (half a batch)
EC = D // P  # contraction chunks (8)
NQB = 8      # query blocks per core
NKB = S // P  # key blocks per batch (16)
NCORES = 8
GROUPS = [[0, 1], [2, 3], [4, 5], [6, 7]]  # batch-pair replica groups
MASKV = -960.0  # additive pre-scale mask; -30 after the 1/sqrt(D) scale

_compiled_nc = None
_runner = None  # cached (sharded_jit, in_names, out_names, out_avals, n_params)
last_result = None  # kept for compatibility with older test harnesses


def _trace_kernel(tc, out, xT, xhT, xqT, wqT, wkT, wvT, maskadd):
    nc = tc.nc
    f32 = mybir.dt.float32
    bf16 = mybir.dt.bfloat16
    ts = bass.ts

    with (
        tc.tile_pool(name="sb", bufs=1) as sb,
        tc.tile_pool(name="dram", bufs=1, space="DRAM") as dram,
    ):
        # ---- persistent SBUF ----
        xhT_s = sb.tile([P, EC, H], bf16)   # x^T columns of own key half
        xqT_s = sb.tile([P, EC, D], bf16)   # own-query columns of x^T
        KT_s = sb.tile([P, EC, S], bf16)    # K^T full (d on partitions)
        V_s = sb.tile([P, NKB, D], bf16)    # V full, natural (s on partitions)
        QT_s = sb.tile([P, EC, D], bf16)    # Q^T for own queries
        mask_s = sb.tile([P, 2 * P], f32)   # additive mask, last 2 key tiles

        # Only K^T is exchanged: a pair-wise AllGather costs ~45us on the
        # serialized CC ring (12us trigger latency + slow transport), so a
        # second (V) gather could never land before the first AV phase. V is
        # projected in full on both cores instead.
        kin = dram.tile([EC, P, H], bf16)       # own K^T half  [d, s_local]
        kout = dram.tile([2, EC, P, H], bf16)   # gathered K^T

        # weights ring: wk -> slot0, wq -> slot1, wv -> slot0 (after the K
        # projection's last read of wk)
        def load_w(w_dram, nm):
            w_s = sb.tile([P, EC, D], bf16, tag="w", bufs=2, name=nm)
            for ec in range(EC):
                nc.sync.dma_start(w_s[:, ec], w_dram[ts(ec, P), :])
            return w_s

        # ---- input loads (sync queues are FIFO, so bulk loads are staged
        # between the stores they must not delay) ----
        # wk + xhT interleaved ec-major so the ec-outer K matmuls stream
        # right behind the DMA during the load-bound first ~12us.
        # load halves in exact consumption order of the 4 K chunks
        # (wk-lo+xhT-lo paced at ~0.75us/ec-chunk vs 0.86us of PE work, so
        # the K projection is PE-bound from the second chunk on)
        wk_s = sb.tile([P, EC, D], bf16, tag="w", bufs=2, name="wk_s")
        for ec in range(EC):
            nc.sync.dma_start(wk_s[:, ec, :512], wkT[ts(ec, P), :512])
            nc.sync.dma_start(xhT_s[:, ec, :512], xhT[ts(ec, P), :512])
        for ec in range(EC):
            nc.sync.dma_start(wk_s[:, ec, 512:], wkT[ts(ec, P), 512:])
        for ec in range(EC):
            nc.sync.dma_start(xhT_s[:, ec, 512:], xhT[ts(ec, P), 512:])
        nc.sync.dma_start(mask_s, maskadd)

        # One PSUM pool for the whole kernel ("s" ring 4 banks + "big" ring
        # 4 banks): closing a scoped pool mid-kernel acts as a coarse
        # barrier on every pending accumulator copy (~10us PE stall).
        with tc.tile_pool(name="ps", bufs=2, space="PSUM") as ps:
            # ---- K^T half projection: KTh[d,s] = sum_e wkT[e,d]*xhT[e,s] ----
            # ec-outer over 4 concurrent PSUM groups: each arriving (wk, xhT)
            # ec-chunk feeds 4 matmuls, so the PE streams behind the initial
            # DMA instead of idling ~7us.
            def k_chunk(sc, dh):
                kaccs = [ps.tile([P, 512], f32, tag="s", bufs=4,
                                 name=f"kb{sc}_{dh}_{i}") for i in range(4)]
                for ec in range(EC):
                    for i in range(4):
                        nc.tensor.matmul(
                            kaccs[i], wk_s[:, ec, ts(4 * dh + i, P)],
                            xhT_s[:, ec, ts(sc, 512)],
                            start=(ec == 0), stop=(ec == EC - 1))
                for i in range(4):
                    kst = sb.tile([P, 512], bf16, tag="kst", bufs=3)
                    nc.scalar.copy(kst, kaccs[i])
                    nc.scalar.dma_start(kin[4 * dh + i, :, ts(sc, 512)], kst)

            # QT-phase loads issue before the K chunks run; the kin stores
            # go out on the scalar engine's queue so they never sit behind
            # these bulk loads
            wq_s = load_w(wqT, "wq_s")
            for ec in range(EC):
                nc.sync.dma_start(xqT_s[:, ec], xqT[ts(ec, P), :])
            k_chunk(0, 0)
            k_chunk(0, 1)
            k_chunk(1, 0)
            k_chunk(1, 1)

            nc.gpsimd.collective_compute(
                "AllGather", mybir.AluOpType.bypass, replica_groups=GROUPS,
                ins=[kin[:]], outs=[kout[:]])

            # V-phase loads: x^T streamed in 4 column-quads through a ring
            # (the V projection is done on both cores of a pair -- cheaper
            # than a second collective). Quad loads 2+ stall on the ring
            # until the V loop frees a slot; that also paces the rbK stall.
            wv_s = load_w(wvT, "wv_s")
            xv_t = [None] * 4

            def load_xv(q):
                xv = sb.tile([P, EC, 512], bf16, tag="xv", bufs=2,
                             name=f"xv{q}")
                for ec in range(EC):
                    nc.sync.dma_start(xv[:, ec],
                                      xT[ts(ec, P), 512 * q:512 * (q + 1)])
                xv_t[q] = xv

            # quads 0/1 preloaded here; 2/3 are traced inside the V loop so
            # the ring reuse (2 bufs) serializes correctly behind the reads
            load_xv(0)
            load_xv(1)

            # readback: stalls sync until ccK completes (~85us); the hoisted
            # s_phases below need K^T only from ~120us
            for g in range(2):
                for dc in range(EC):
                    nc.sync.dma_start(KT_s[:, dc, g * H:(g + 1) * H],
                                      kout[g, dc])

            # ---- Q^T projection: QT[d, q] = sum_e WqT[e, d] * xqT[e, q] ----
            for dc in range(EC):
                for nh in range(2):
                    acch = ps.tile([P, 512], f32, tag="s", bufs=4)
                    for ec in range(EC):
                        nc.tensor.matmul(
                            acch, wq_s[:, ec, ts(dc, P)],
                            xqT_s[:, ec, ts(nh, 512)],
                            start=(ec == 0), stop=(ec == EC - 1))
                    nc.scalar.copy(QT_s[:, dc, ts(nh, 512)], acch)

            # ---- attention, one 128-row query block at a time ----
            # Software-pipelined: S/exp of the NEXT block is traced between the
            # S/exp and transpose/AV of the current one, so the PE has matmul
            # work while ACT/DVE chew through exp and P^T copies.
            inv_sqrt_d = 1.0 / float(np.sqrt(D))

            def s_phase(j):
                nkt = 2 * j + 2          # key tiles (uniform across cores)
                ncols = nkt * P
                nch = (ncols + 511) // 512
                p_sb = sb.tile([P, S], bf16, tag="p_sb", bufs=3)
                pT_sb = sb.tile([P, NKB, P], bf16, tag="pT_sb", bufs=3)
                dsl = sb.tile([P, 4], f32, tag="dsl", bufs=3)
                for ch in range(nch):
                    c0 = ch * 512
                    cw = min(512, ncols - c0)
                    sfull = ps.tile([P, 512], f32, tag="s", bufs=4)
                    sps = sfull[:, :cw]
                    for dc in range(EC):
                        nc.tensor.matmul(
                            sps, QT_s[:, dc, ts(j, P)], KT_s[:, dc, c0:c0 + cw],
                            start=(dc == 0), stop=(dc == EC - 1))
                    if c0 + cw == ncols:  # last chunk holds the 2 maskable tiles
                        nc.vector.tensor_add(
                            sps[:, cw - 2 * P:cw], sps[:, cw - 2 * P:cw], mask_s)
                    nc.scalar.activation(
                        p_sb[:, c0:c0 + cw], sps,
                        mybir.ActivationFunctionType.Exp,
                        scale=inv_sqrt_d,
                        accum_out=dsl[:, ch:ch + 1])
                    # xbar-transpose the finished chunk off the hot engines:
                    # pT_sb[p, kt, q] = p_sb[q, 128*kt + p]
                    nc.sync.dma_start(pT_sb[:, ch * 4:ch * 4 + cw // P, :],
                                      p_sb[:, c0:c0 + cw], transpose=True)
                return p_sb, pT_sb, dsl, nkt, nch

            def av_phase(j, p_sb, pT_sb, dsl, nkt, nch, nsplit=2):
                denom = sb.tile([P, 1], f32, tag="den", bufs=2)
                nc.vector.reduce_sum(denom, dsl[:, :nch], axis=mybir.AxisListType.X)
                recip = sb.tile([P, 1], f32, tag="rcp", bufs=2)
                nc.vector.reciprocal(recip, denom)

                acc = ps.tile([P, D], f32, tag="big")
                for kt in range(nkt):
                    for nh in range(2):
                        nc.tensor.matmul(
                            acc[:, ts(nh, 512)], pT_sb[:, kt, :],
                            V_s[:, kt, ts(nh, 512)],
                            start=(kt == 0), stop=(kt == nkt - 1))
                o_sb = sb.tile([P, D], f32, tag="o_sb", bufs=2)
                # normalize on DVE (idle now), split so the out DMA overlaps
                w = D // nsplit
                for i in range(nsplit):
                    nc.vector.tensor_scalar_mul(
                        o_sb[:, i * w:(i + 1) * w], acc[:, i * w:(i + 1) * w],
                        recip)
                    nc.sync.dma_start(out[j, :, i * w:(i + 1) * w],
                                      o_sb[:, i * w:(i + 1) * w])

            # ---- V projection (full batch, duplicated on the pair) ----
            # V[s, d] = sum_e xT[e, s] * WvT[e, d]; the two biggest s_phases
            # are hoisted into the tail of this loop (after the K^T readback
            # has landed) so their exp/transpose latencies hide under V
            # matmuls and the first av_phase can start right at V's end.
            v_hoist = {}
            for kb in range(NKB):
                if kb == 4:
                    load_xv(2)
                elif kb == 8:
                    load_xv(3)
                xv = xv_t[kb // 4]
                acc = ps.tile([P, D], f32, tag="big")
                for ec in range(EC):
                    lhsT = xv[:, ec, ts(kb % 4, P)]
                    for nh in range(2):
                        nc.tensor.matmul(
                            acc[:, ts(nh, 512)], lhsT, wv_s[:, ec, ts(nh, 512)],
                            start=(ec == 0), stop=(ec == EC - 1))
                nc.vector.tensor_copy(V_s[:, kb], acc)
                if kb == 12:
                    v_hoist[7] = s_phase(7)
                elif kb == 14:
                    v_hoist[6] = s_phase(6)

            # big first; small blocks interleaved late, with the tail run
            # 2 s_phases ahead (3-deep p/pT rings) so the last av-phases
            # never wait on an exp->transpose chain
            states = dict(v_hoist)

            def run_s(j):
                states[j] = s_phase(j)

            def run_av(j, nsplit=2):
                av_phase(j, *states.pop(j), nsplit=nsplit)

            run_av(7); run_s(5); run_av(6); run_s(0); run_av(5); run_s(1)
            run_av(0); run_s(2); run_s(3); run_av(1); run_s(4); run_av(2)
            run_av(3); run_av(4, nsplit=4)


def build_nc(debug=False):
    nc = bacc.Bacc("TRN2", target_bir_lowering=False, debug=debug,
                   enable_asserts=False, num_devices=NCORES)
    bf16 = mybir.dt.bfloat16
    f32 = mybir.dt.float32
    xT = nc.dram_tensor("xT", (D, S), bf16, kind="ExternalInput").ap()
    xhT = nc.dram_tensor("xhT", (D, H), bf16, kind="ExternalInput").ap()
    xqT = nc.dram_tensor("xqT", (D, D), bf16, kind="ExternalInput").ap()
    wqT = nc.dram_tensor("wqT", (D, D), bf16, kind="ExternalInput").ap()
    wkT = nc.dram_tensor("wkT", (D, D), bf16, kind="ExternalInput").ap()
    wvT = nc.dram_tensor("wvT", (D, D), bf16, kind="ExternalInput").ap()
    maskadd = nc.dram_tensor("maskadd", (P, 2 * P), f32,
                             kind="ExternalInput").ap()
    out = nc.dram_tensor("out", (NQB, P, D), f32, kind="ExternalOutput").ap()
    with tile.TileContext(nc) as tc:
        _trace_kernel(tc, out, xT, xhT, xqT, wqT, wkT, wvT, maskadd)
    nc.compile()
    return nc


def _get_compiled():
    global _compiled_nc
    if _compiled_nc is None:
        _compiled_nc = build_nc(debug=False)
    return _compiled_nc


def _get_runner():
    """Jit-once shard_map runner over the 8 NeuronCores.

    Mirrors bass2jax.run_bass_via_pjrt's multi-core branch, but caches the
    jitted executable so repeat kernel() calls skip retracing/recompiling.
    """
    global _runner
    if _runner is not None:
        return _runner
    import jax
    from jax.experimental.shard_map import shard_map
    from jax.sharding import Mesh, PartitionSpec
    from concourse import bass2jax

    nc = _get_compiled()
    bass2jax.install_neuronx_cc_hook()

    partition_name = (nc.partition_id_tensor.name
                      if nc.partition_id_tensor else None)
    in_names, out_names, out_avals, zero_outs = [], [], [], []
    for alloc in nc.m.functions[0].allocations:
        if not isinstance(alloc, mybir.MemoryLocationSet):
            continue
        name = alloc.memorylocations[0].name
        if alloc.kind == "ExternalInput":
            if name != partition_name:
                in_names.append(name)
        elif alloc.kind == "ExternalOutput":
            shape = tuple(alloc.tensor_shape)
            dtype = mybir.dt.np(alloc.dtype)
            out_names.append(name)
            out_avals.append(jax.core.ShapedArray(shape, dtype))
            zero_outs.append(np.zeros(shape, dtype))
    n_params = len(in_names)
    all_in_names = list(in_names) + list(out_names)
    if partition_name is not None:
        all_in_names.append(partition_name)
    donate = tuple(range(n_params, n_params + len(out_names)))

    def _body(*args):
        operands = list(args)
        if partition_name is not None:
            operands.append(bass2jax.partition_id_tensor())
        outs = bass2jax._bass_exec_p.bind(
            *operands,
            out_avals=tuple(out_avals),
            in_names=tuple(all_in_names),
            out_names=tuple(out_names),
            lowering_input_output_aliases=(),
            sim_require_finite=True,
            sim_require_nnan=True,
            nc=nc,
        )
        return tuple(outs)

    devices = jax.devices()[:NCORES]
    mesh = Mesh(np.asarray(devices), ("core",))
    nin = n_params + len(out_names)
    sharded = jax.jit(
        shard_map(_body, mesh=mesh,
                  in_specs=(PartitionSpec("core"),) * nin,
                  out_specs=(PartitionSpec("core"),) * len(out_names),
                  check_rep=False),
        donate_argnums=donate, keep_unused=True)
    _runner = (sharded, in_names, out_names, out_avals, n_params, zero_outs, mesh)
    return _runner


def run_device(in_maps):
    """Execute the compiled NEFF on all 8 cores; returns per-core output dicts."""
    sharded, in_names, out_names, out_avals, n_params, zero_outs, _ = _get_runner()
    concat_in = [
        np.concatenate([np.asarray(in_maps[c][nm]) for c in range(NCORES)], axis=0)
        for nm in in_names
    ]
    concat_zeros = [
        np.zeros((NCORES * z.shape[0], *z.shape[1:]), z.dtype) for z in zero_outs
    ]
    out_arrs = sharded(*concat_in, *concat_zeros)
    return [
        {nm: np.asarray(out_arrs[i]).reshape(NCORES, *out_avals[i].shape)[c]
         for i, nm in enumerate(out_names)}
        for c in range(NCORES)
    ]


def make_in_maps(x):
    """Per-core host-side slicing + layout prep (no matmul math here)."""
    x = np.asarray(x, dtype=np.float32)
    r = np.arange(P)
    tri_add = np.where(r[None, :] <= r[:, None], 0.0, MASKV).astype(np.float32)
    mask_h = []
    for h in range(2):
        if h == 0:
            blk = np.concatenate(
                [tri_add, np.full((P, P), MASKV, np.float32)], axis=1)
        else:
            blk = np.concatenate([np.zeros((P, P), np.float32), tri_add], axis=1)
        mask_h.append(np.ascontiguousarray(blk).astype(np.float32))

    in_maps = []
    xT_b = {}
    for c in range(NCORES):
        b, h = c // 2, c % 2
        if b not in xT_b:
            xT_b[b] = np.ascontiguousarray(x[b].T).astype(BF16)
        blocks = [2 * j + h for j in range(NQB)]
        xq = np.concatenate([x[b][g * P:(g + 1) * P] for g in blocks], axis=0)
        xqT = np.ascontiguousarray(xq.T).astype(BF16)
        in_maps.append({
            "xT": xT_b[b],
            "xhT": np.ascontiguousarray(xT_b[b][:, h * H:(h + 1) * H]),
            "xqT": xqT,
            "maskadd": mask_h[h],
        })
    return in_maps


def make_weight_map(inputs):
    """Pre-transposed bf16 weights keyed by NEFF input name."""
    return {
        "wqT": np.ascontiguousarray(np.asarray(inputs["Wq"], np.float32).T).astype(BF16),
        "wkT": np.ascontiguousarray(np.asarray(inputs["Wk"], np.float32).T).astype(BF16),
        "wvT": np.ascontiguousarray(np.asarray(inputs["Wv"], np.float32).T).astype(BF16),
    }


def kernel(x, Wq, bq, Wk, bk, Wv, bv, mask):
    global last_result
    x = np.asarray(x, np.float32)
    Wq = np.asarray(Wq, np.float32)
    Wk = np.asarray(Wk, np.float32)
    Wv = np.asarray(Wv, np.float32)
    bq = np.asarray(bq, np.float32)
    bk = np.asarray(bk, np.float32)
    bv = np.asarray(bv, np.float32)
    mask = np.asarray(mask)

    causal = bool(np.array_equal(mask != 0, np.tril(np.ones(mask.shape, bool))))
    if np.any(bq) or np.any(bk) or not causal:
        return _np_reference(x, Wq, bq, Wk, bk, Wv, bv, mask)

    in_maps = make_in_maps(x)
    wT = {
        "wqT": np.ascontiguousarray(Wq.T).astype(BF16),
        "wkT": np.ascontiguousarray(Wk.T).astype(BF16),
        "wvT": np.ascontiguousarray(Wv.T).astype(BF16),
    }
    for m in in_maps:
        m.update(wT)

    results = None
    for attempt in range(3):  # remote NeuronCores occasionally wedge transiently
        try:
            results = run_device(in_maps)
            break
        except Exception:
            if attempt == 2:
                raise
            time.sleep(30)

    out = np.empty((B * S, D), np.float32)
    for c in range(NCORES):
        b, h = c // 2, c % 2
        o = np.asarray(results[c]["out"], np.float32)
        for j in range(NQB):
            g = 2 * j + h
            out[b * S + g * P: b * S + (g + 1) * P] = o[j]
    if np.any(bv):
        out = out + bv[None, :]  # attn rows sum to 1, so bv adds exactly
    return out


def _np_reference(x, Wq, bq, Wk, bk, Wv, bv, mask):
    outs = []
    for b in range(x.shape[0]):
        xb = x[b]
        Q = xb @ Wq.T + bq
        K = xb @ Wk.T + bk
        V = xb @ Wv.T + bv
        Sc = (Q @ K.T) / np.float32(np.sqrt(x.shape[2]))
        Sc = np.where(mask == 0, np.float32(-1e9), Sc)
        Sc = Sc - Sc.max(axis=1, keepdims=True)
        E = np.exp(Sc)
        A = E / E.sum(axis=1, keepdims=True)
        outs.append(A @ V)
    return np.concatenate(outs, axis=0).astype(np.float32)


# revision 24
# speedup vs baseline: 1.0536x; 1.0054x over previous
"""Causal self-attention (B=4, S=2048, D=1024, single 1024-wide head) on 8 TRN2 cores.

Sharding: core c -> batch b=c//2, parity h=c%2. The two cores of a batch
split the K^T projection by key half (core h computes keys [1024h,
1024h+1024)) and exchange halves with one pair-wise HBM AllGather, halving
the K projection work. V is projected in full on both cores: a collective
costs ~45us on the serialized CC ring (12us trigger latency + slow
transport), so a second gather could never land before the first AV phase
needs V. Each core handles the 8 query blocks {h, h+2, ..., h+14} (128 rows
each); key-extents padded to 256*(j+1) make the program identical on every
core; causality differences live in per-core additive-mask input data, not
control flow.

Phases: K^T-half projection (ec-outer over 4 concurrent PSUM groups so the
PE streams behind the initial input DMA) -> trigger gather-K -> Q^T
projection -> V projection (x^T streamed through a 2-buffer ring; the two
biggest s_phases hoisted into its tail) -> attention. Queue discipline
matters: DMA queues are FIFO, so the K bounce stores ride the scalar
engine's queue (never stuck behind bulk loads on sync), bulk loads are
issued in consumption order, and the gather readback's sync stall sits
where nothing later on that queue is needed sooner.

All matmuls run on the PE in bf16 with fp32 PSUM accumulation. Softmax skips
max-subtraction (scores are ~N(0,1); exp stays in fp32 range) so the
denominator comes free from the Exp activation's accumulate output. The
attention tail runs 2 s_phases ahead of the av_phases so the last av never
waits on an exp->transpose chain.
"""

import time

import numpy as np
import ml_dtypes

import concourse.bass as bass
import concourse.bacc as bacc
import concourse.tile as tile
from concourse import mybir
from concourse import bass_utils

BF16 = ml_dtypes.bfloat16
P = 128
B, S, D = 4, 2048, 1024
H = S // 2   # keys owned per core (half a batch)
EC = D // P  # contraction chunks (8)
NQB = 8      # query blocks per core
NKB = S // P  # key blocks per batch (16)
NCORES = 8
GROUPS = [[0, 1], [2, 3], [4, 5], [6, 7]]  # batch-pair replica groups
MASKV = -960.0  # additive pre-scale mask; -30 after the 1/sqrt(D) scale

_compiled_nc = None
_runner = None  # cached (sharded_jit, in_names, out_names, out_avals, n_params)
last_result = None  # kept for compatibility with older test harnesses


def _trace_kernel(tc, out, xT, xhT, xqT, wqT, wkT, wvT, maskadd):
    nc = tc.nc
    f32 = mybir.dt.float32
    bf16 = mybir.dt.bfloat16
    ts = bass.ts

    with (
        tc.tile_pool(name="sb", bufs=1) as sb,
        tc.tile_pool(name="dram", bufs=1, space="DRAM") as dram,
    ):
        # ---- persistent SBUF ----
        xhT_s = sb.tile([P, EC, H], bf16)   # x^T columns of own key half
        xqT_s = sb.tile([P, EC, D], bf16)   # own-query columns of x^T
        KT_s = sb.tile([P, EC, S], bf16)    # K^T full (d on partitions)
        V_s = sb.tile([P, NKB, D], bf16)    # V full, natural (s on partitions)
        QT_s = sb.tile([P, EC, D], bf16)    # Q^T for own queries
        mask_s = sb.tile([P, 2 * P], f32)   # additive mask, last 2 key tiles

        # Only K^T is exchanged: a pair-wise AllGather costs ~45us on the
        # serialized CC ring (12us trigger latency + slow transport), so a
        # second (V) gather could never land before the first AV phase. V is
        # projected in full on both cores instead.
        kin = dram.tile([EC, P, H], bf16)       # own K^T half  [d, s_local]
        kout = dram.tile([2, EC, P, H], bf16)   # gathered K^T

        # weights ring: wk -> slot0, wq -> slot1, wv -> slot0 (after the K
        # projection's last read of wk)
        def load_w(w_dram, nm):
            w_s = sb.tile([P, EC, D], bf16, tag="w", bufs=2, name=nm)
            for ec in range(EC):
                nc.sync.dma_start(w_s[:, ec], w_dram[ts(ec, P), :])
            return w_s

        # ---- input loads (sync queues are FIFO, so bulk loads are staged
        # between the stores they must not delay) ----
        # wk + xhT interleaved ec-major so the ec-outer K matmuls stream
        # right behind the DMA during the load-bound first ~12us.
        # load halves in exact consumption order of the 4 K chunks
        # (wk-lo+xhT-lo paced at ~0.75us/ec-chunk vs 0.86us of PE work, so
        # the K projection is PE-bound from the second chunk on)
        wk_s = sb.tile([P, EC, D], bf16, tag="w", bufs=2, name="wk_s")
        for ec in range(EC):
            nc.sync.dma_start(wk_s[:, ec, :512], wkT[ts(ec, P), :512])
            nc.sync.dma_start(xhT_s[:, ec, :512], xhT[ts(ec, P), :512])
        for ec in range(EC):
            nc.sync.dma_start(wk_s[:, ec, 512:], wkT[ts(ec, P), 512:])
        for ec in range(EC):
            nc.sync.dma_start(xhT_s[:, ec, 512:], xhT[ts(ec, P), 512:])
        nc.sync.dma_start(mask_s, maskadd)

        # One PSUM pool for the whole kernel ("s" ring 4 banks + "big" ring
        # 4 banks): closing a scoped pool mid-kernel acts as a coarse
        # barrier on every pending accumulator copy (~10us PE stall).
        with tc.tile_pool(name="ps", bufs=2, space="PSUM") as ps:
            # ---- K^T half projection: KTh[d,s] = sum_e wkT[e,d]*xhT[e,s] ----
            # ec-outer over 4 concurrent PSUM groups: each arriving (wk, xhT)
            # ec-chunk feeds 4 matmuls, so the PE streams behind the initial
            # DMA instead of idling ~7us.
            def k_chunk(sc, dh, use_big=False):
                if use_big:
                    # two accumulation groups per [P, D] tile (separate PSUM
                    # banks) so adjacent chunks never share ring slots and a
                    # chunk boundary never stalls on the prior chunk's copies
                    bigs = [ps.tile([P, D], f32, tag="big",
                                    name=f"kB{sc}_{dh}_{i}") for i in range(2)]
                    kaccs = [bigs[0][:, :512], bigs[0][:, 512:],
                             bigs[1][:, :512], bigs[1][:, 512:]]
                else:
                    kaccs = [ps.tile([P, 512], f32, tag="s", bufs=4,
                                     name=f"kb{sc}_{dh}_{i}") for i in range(4)]
                for ec in range(EC):
                    for i in range(4):
                        nc.tensor.matmul(
                            kaccs[i], wk_s[:, ec, ts(4 * dh + i, P)],
                            xhT_s[:, ec, ts(sc, 512)],
                            start=(ec == 0), stop=(ec == EC - 1))
                for i in range(4):
                    kst = sb.tile([P, 512], bf16, tag="kst", bufs=3)
                    nc.scalar.copy(kst, kaccs[i])
                    nc.scalar.dma_start(kin[4 * dh + i, :, ts(sc, 512)], kst)

            # QT-phase loads issue before the K chunks run; the kin stores
            # go out on the scalar engine's queue so they never sit behind
            # these bulk loads
            wq_s = load_w(wqT, "wq_s")
            for ec in range(EC):
                nc.sync.dma_start(xqT_s[:, ec], xqT[ts(ec, P), :])
            k_chunk(0, 0)
            k_chunk(0, 1, use_big=True)
            k_chunk(1, 0)
            k_chunk(1, 1, use_big=True)

            nc.gpsimd.collective_compute(
                "AllGather", mybir.AluOpType.bypass, replica_groups=GROUPS,
                ins=[kin[:]], outs=[kout[:]])

            # V-phase loads: x^T streamed in 4 column-quads through a ring
            # (the V projection is done on both cores of a pair -- cheaper
            # than a second collective). Quad loads 2+ stall on the ring
            # until the V loop frees a slot; that also paces the rbK stall.
            wv_s = load_w(wvT, "wv_s")
            xv_t = [None] * 4

            def load_xv(q):
                xv = sb.tile([P, EC, 512], bf16, tag="xv", bufs=2,
                             name=f"xv{q}")
                for ec in range(EC):
                    nc.sync.dma_start(xv[:, ec],
                                      xT[ts(ec, P), 512 * q:512 * (q + 1)])
                xv_t[q] = xv

            # quads 0/1 preloaded here; 2/3 are traced inside the V loop so
            # the ring reuse (2 bufs) serializes correctly behind the reads
            load_xv(0)
            load_xv(1)

            # readback: stalls sync until ccK completes (~85us); the hoisted
            # s_phases below need K^T only from ~120us
            for g in range(2):
                for dc in range(EC):
                    nc.sync.dma_start(KT_s[:, dc, g * H:(g + 1) * H],
                                      kout[g, dc])

            # ---- Q^T projection: QT[d, q] = sum_e WqT[e, d] * xqT[e, q] ----
            for dc in range(EC):
                for nh in range(2):
                    acch = ps.tile([P, 512], f32, tag="s", bufs=4)
                    for ec in range(EC):
                        nc.tensor.matmul(
                            acch, wq_s[:, ec, ts(dc, P)],
                            xqT_s[:, ec, ts(nh, 512)],
                            start=(ec == 0), stop=(ec == EC - 1))
                    nc.scalar.copy(QT_s[:, dc, ts(nh, 512)], acch)

            # ---- attention, one 128-row query block at a time ----
            # Software-pipelined: S/exp of the NEXT block is traced between the
            # S/exp and transpose/AV of the current one, so the PE has matmul
            # work while ACT/DVE chew through exp and P^T copies.
            inv_sqrt_d = 1.0 / float(np.sqrt(D))

            def s_phase(j):
                nkt = 2 * j + 2          # key tiles (uniform across cores)
                ncols = nkt * P
                nch = (ncols + 511) // 512
                p_sb = sb.tile([P, S], bf16, tag="p_sb", bufs=3)
                pT_sb = sb.tile([P, NKB, P], bf16, tag="pT_sb", bufs=3)
                dsl = sb.tile([P, 4], f32, tag="dsl", bufs=3)
                for ch in range(nch):
                    c0 = ch * 512
                    cw = min(512, ncols - c0)
                    sfull = ps.tile([P, 512], f32, tag="s", bufs=4)
                    sps = sfull[:, :cw]
                    for dc in range(EC):
                        nc.tensor.matmul(
                            sps, QT_s[:, dc, ts(j, P)], KT_s[:, dc, c0:c0 + cw],
                            start=(dc == 0), stop=(dc == EC - 1))
                    if c0 + cw == ncols:  # last chunk holds the 2 maskable tiles
                        nc.vector.tensor_add(
                            sps[:, cw - 2 * P:cw], sps[:, cw - 2 * P:cw], mask_s)
                    nc.scalar.activation(
                        p_sb[:, c0:c0 + cw], sps,
                        mybir.ActivationFunctionType.Exp,
                        scale=inv_sqrt_d,
                        accum_out=dsl[:, ch:ch + 1])
                    # xbar-transpose the finished chunk off the hot engines:
                    # pT_sb[p, kt, q] = p_sb[q, 128*kt + p]
                    nc.sync.dma_start(pT_sb[:, ch * 4:ch * 4 + cw // P, :],
                                      p_sb[:, c0:c0 + cw], transpose=True)
                return p_sb, pT_sb, dsl, nkt, nch

            def av_phase(j, p_sb, pT_sb, dsl, nkt, nch, nsplit=2):
                denom = sb.tile([P, 1], f32, tag="den", bufs=2)
                nc.vector.reduce_sum(denom, dsl[:, :nch], axis=mybir.AxisListType.X)
                recip = sb.tile([P, 1], f32, tag="rcp", bufs=2)
                nc.vector.reciprocal(recip, denom)

                acc = ps.tile([P, D], f32, tag="big")
                for kt in range(nkt):
                    for nh in range(2):
                        nc.tensor.matmul(
                            acc[:, ts(nh, 512)], pT_sb[:, kt, :],
                            V_s[:, kt, ts(nh, 512)],
                            start=(kt == 0), stop=(kt == nkt - 1))
                o_sb = sb.tile([P, D], f32, tag="o_sb", bufs=2)
                # normalize on DVE (idle now), split so the out DMA overlaps
                w = D // nsplit
                for i in range(nsplit):
                    nc.vector.tensor_scalar_mul(
                        o_sb[:, i * w:(i + 1) * w], acc[:, i * w:(i + 1) * w],
                        recip)
                    nc.sync.dma_start(out[j, :, i * w:(i + 1) * w],
                                      o_sb[:, i * w:(i + 1) * w])

            # ---- V projection (full batch, duplicated on the pair) ----
            # V[s, d] = sum_e xT[e, s] * WvT[e, d]; the two biggest s_phases
            # are hoisted into the tail of this loop (after the K^T readback
            # has landed) so their exp/transpose latencies hide under V
            # matmuls and the first av_phase can start right at V's end.
            v_hoist = {}
            for kb in range(NKB):
                if kb == 4:
                    load_xv(2)
                elif kb == 8:
                    load_xv(3)
                xv = xv_t[kb // 4]
                acc = ps.tile([P, D], f32, tag="big")
                for ec in range(EC):
                    lhsT = xv[:, ec, ts(kb % 4, P)]
                    for nh in range(2):
                        nc.tensor.matmul(
                            acc[:, ts(nh, 512)], lhsT, wv_s[:, ec, ts(nh, 512)],
                            start=(ec == 0), stop=(ec == EC - 1))
                nc.vector.tensor_copy(V_s[:, kb], acc)
                if kb == 12:
                    v_hoist[7] = s_phase(7)
                elif kb == 14:
                    v_hoist[6] = s_phase(6)

            # big first; small blocks interleaved late, with the tail run
            # 2 s_phases ahead (3-deep p/pT rings) so the last av-phases
            # never wait on an exp->transpose chain
            states = dict(v_hoist)

            def run_s(j):
                states[j] = s_phase(j)

            def run_av(j, nsplit=2):
                av_phase(j, *states.pop(j), nsplit=nsplit)

            run_av(7); run_s(5); run_av(6); run_s(0); run_av(5); run_s(1)
            run_av(0); run_s(2); run_s(3); run_av(1); run_s(4); run_av(2)
            run_av(3); run_av(4, nsplit=4)


def build_nc(debug=False):
    nc = bacc.Bacc("TRN2", target_bir_lowering=False, debug=debug,
                   enable_asserts=False, num_devices=NCORES)
    bf16 = mybir.dt.bfloat16
    f32 = mybir.dt.float32
    xT = nc.dram_tensor("xT", (D, S), bf16, kind="ExternalInput").ap()
    xhT = nc.dram_tensor("xhT", (D, H), bf16, kind="ExternalInput").ap()
    xqT = nc.dram_tensor("xqT", (D, D), bf16, kind="ExternalInput").ap()
    wqT = nc.dram_tensor("wqT", (D, D), bf16, kind="ExternalInput").ap()
    wkT = nc.dram_tensor("wkT", (D, D), bf16, kind="ExternalInput").ap()
    wvT = nc.dram_tensor("wvT", (D, D), bf16, kind="ExternalInput").ap()
    maskadd = nc.dram_tensor("maskadd", (P, 2 * P), f32,
                             kind="ExternalInput").ap()
    out = nc.dram_tensor("out", (NQB, P, D), f32, kind="ExternalOutput").ap()
    with tile.TileContext(nc) as tc:
        _trace_kernel(tc, out, xT, xhT, xqT, wqT, wkT, wvT, maskadd)
    nc.compile()
    return nc


def _get_compiled():
    global _compiled_nc
    if _compiled_nc is None:
        _compiled_nc = build_nc(debug=False)
    return _compiled_nc


def _get_runner():
    """Jit-once shard_map runner over the 8 NeuronCores.

    Mirrors bass2jax.run_bass_via_pjrt's multi-core branch, but caches the
    jitted executable so repeat kernel() calls skip retracing/recompiling.
    """
    global _runner
    if _runner is not None:
        return _runner
    import jax
    from jax.experimental.shard_map import shard_map
    from jax.sharding import Mesh, PartitionSpec
    from concourse import bass2jax

    nc = _get_compiled()
    bass2jax.install_neuronx_cc_hook()

    partition_name = (nc.partition_id_tensor.name
                      if nc.partition_id_tensor else None)
    in_names, out_names, out_avals, zero_outs = [], [], [], []
    for alloc in nc.m.functions[0].allocations:
        if not isinstance(alloc, mybir.MemoryLocationSet):
            continue
        name = alloc.memorylocations[0].name
        if alloc.kind == "ExternalInput":
            if name != partition_name:
                in_names.append(name)
        elif alloc.kind == "ExternalOutput":
            shape = tuple(alloc.tensor_shape)
            dtype = mybir.dt.np(alloc.dtype)
            out_names.append(name)
            out_avals.append(jax.core.ShapedArray(shape, dtype))
            zero_outs.append(np.zeros(shape, dtype))
    n_params = len(in_names)
    all_in_names = list(in_names) + list(out_names)
    if partition_name is not None:
        all_in_names.append(partition_name)
    donate = tuple(range(n_params, n_params + len(out_names)))

    def _body(*args):
        operands = list(args)
        if partition_name is not None:
            operands.append(bass2jax.partition_id_tensor())
        outs = bass2jax._bass_exec_p.bind(
            *operands,
            out_avals=tuple(out_avals),
            in_names=tuple(all_in_names),
            out_names=tuple(out_names),
            lowering_input_output_aliases=(),
            sim_require_finite=True,
            sim_require_nnan=True,
            nc=nc,
        )
        return tuple(outs)

    devices = jax.devices()[:NCORES]
    mesh = Mesh(np.asarray(devices), ("core",))
    nin = n_params + len(out_names)
    sharded = jax.jit(
        shard_map(_body, mesh=mesh,
                  in_specs=(PartitionSpec("core"),) * nin,
                  out_specs=(PartitionSpec("core"),) * len(out_names),
                  check_rep=False),
        donate_argnums=donate, keep_unused=True)
    _runner = (sharded, in_names, out_names, out_avals, n_params, zero_outs, mesh)
    return _runner


def run_device(in_maps):
    """Execute the compiled NEFF on all 8 cores; returns per-core output dicts."""
    sharded, in_names, out_names, out_avals, n_params, zero_outs, _ = _get_runner()
    concat_in = [
        np.concatenate([np.asarray(in_maps[c][nm]) for c in range(NCORES)], axis=0)
        for nm in in_names
    ]
    concat_zeros = [
        np.zeros((NCORES * z.shape[0], *z.shape[1:]), z.dtype) for z in zero_outs
    ]
    out_arrs = sharded(*concat_in, *concat_zeros)
    return [
        {nm: np.asarray(out_arrs[i]).reshape(NCORES, *out_avals[i].shape)[c]
         for i, nm in enumerate(out_names)}
        for c in range(NCORES)
    ]


def make_in_maps(x):
    """Per-core host-side slicing + layout prep (no matmul math here)."""
    x = np.asarray(x, dtype=np.float32)
    r = np.arange(P)
    tri_add = np.where(r[None, :] <= r[:, None], 0.0, MASKV).astype(np.float32)
    mask_h = []
    for h in range(2):
        if h == 0:
            blk = np.concatenate(
                [tri_add, np.full((P, P), MASKV, np.float32)], axis=1)
        else:
            blk = np.concatenate([np.zeros((P, P), np.float32), tri_add], axis=1)
        mask_h.append(np.ascontiguousarray(blk).astype(np.float32))

    in_maps = []
    xT_b = {}
    for c in range(NCORES):
        b, h = c // 2, c % 2
        if b not in xT_b:
            xT_b[b] = np.ascontiguousarray(x[b].T).astype(BF16)
        blocks = [2 * j + h for j in range(NQB)]
        xq = np.concatenate([x[b][g * P:(g + 1) * P] for g in blocks], axis=0)
        xqT = np.ascontiguousarray(xq.T).astype(BF16)
        in_maps.append({
            "xT": xT_b[b],
            "xhT": np.ascontiguousarray(xT_b[b][:, h * H:(h + 1) * H]),
            "xqT": xqT,
            "maskadd": mask_h[h],
        })
    return in_maps


def make_weight_map(inputs):
    """Pre-transposed bf16 weights keyed by NEFF input name."""
    return {
        "wqT": np.ascontiguousarray(np.asarray(inputs["Wq"], np.float32).T).astype(BF16),
        "wkT": np.ascontiguousarray(np.asarray(inputs["Wk"], np.float32).T).astype(BF16),
        "wvT": np.ascontiguousarray(np.asarray(inputs["Wv"], np.float32).T).astype(BF16),
    }


def kernel(x, Wq, bq, Wk, bk, Wv, bv, mask):
    global last_result
    x = np.asarray(x, np.float32)
    Wq = np.asarray(Wq, np.float32)
    Wk = np.asarray(Wk, np.float32)
    Wv = np.asarray(Wv, np.float32)
    bq = np.asarray(bq, np.float32)
    bk = np.asarray(bk, np.float32)
    bv = np.asarray(bv, np.float32)
    mask = np.asarray(mask)

    causal = bool(np.array_equal(mask != 0, np.tril(np.ones(mask.shape, bool))))
    if np.any(bq) or np.any(bk) or not causal:
        return _np_reference(x, Wq, bq, Wk, bk, Wv, bv, mask)

    in_maps = make_in_maps(x)
    wT = {
        "wqT": np.ascontiguousarray(Wq.T).astype(BF16),
        "wkT": np.ascontiguousarray(Wk.T).astype(BF16),
        "wvT": np.ascontiguousarray(Wv.T).astype(BF16),
    }
    for m in in_maps:
        m.update(wT)

    results = None
    for attempt in range(3):  # remote NeuronCores occasionally wedge transiently
        try:
            results = run_device(in_maps)
            break
        except Exception:
            if attempt == 2:
                raise
            time.sleep(30)

    out = np.empty((B * S, D), np.float32)
    for c in range(NCORES):
        b, h = c // 2, c % 2
        o = np.asarray(results[c]["out"], np.float32)
        for j in range(NQB):
            g = 2 * j + h
            out[b * S + g * P: b * S + (g + 1) * P] = o[j]
    if np.any(bv):
        out = out + bv[None, :]  # attn rows sum to 1, so bv adds exactly
    return out


def _np_reference(x, Wq, bq, Wk, bk, Wv, bv, mask):
    outs = []
    for b in range(x.shape[0]):
        xb = x[b]
        Q = xb @ Wq.T + bq
        K = xb @ Wk.T + bk
        V = xb @ Wv.T + bv
        Sc = (Q @ K.T) / np.float32(np.sqrt(x.shape[2]))
        Sc = np.where(mask == 0, np.float32(-1e9), Sc)
        Sc = Sc - Sc.max(axis=1, keepdims=True)
        E = np.exp(Sc)
        A = E / E.sum(axis=1, keepdims=True)
        outs.append(A @ V)
    return np.concatenate(outs, axis=0).astype(np.float32)
